# revision 10
# baseline (speedup 1.0000x reference)
"""Nadaraya-Watson head (retrieval kNN) Trainium2 Bass kernel.

reference:
    dist = ||q - x||_2 over d            (b, s)
    probs = softmax(-dist, axis=s)       (b, s)
    out = probs @ labels                 (b, c)

Strategy (8 NeuronCores, batch-parallel, 8 batches per core):
  dist^2 = sum_d x^2 - 2 q.x + ||q||^2 computed in natural [s=partition,
  d=free] layout so no transposes are needed:
    - DVE: tensor_tensor_reduce(X * (-2q_bcast)) -> per-row -2q.x
    - ACT: activation(Square, accum_out=...)     -> per-row sum x^2
  dist = sqrt via linear seed + 2 Newton-Raphson steps on DVE (avoids the
  sqrt ACT table set; Square/Exp share the exp_and_others set -> one table
  load total).
  Softmax shift is a constant (exact math; values are concentrated around
  dist ~ 22.6 so exp stays in range without a max pass).
  Label reduction: PE matmul with probs column [128,1] stationary and the
  label tile [128,100] moving, accumulated over 64 chunks in PSUM [1,100].
  Normalizer Z via reduce + ones-matmul; final scale by 1/Z on DVE.

The -2q broadcast tile and ||q||^2 are tiny (per-batch) and precomputed on
the host.
"""

from contextlib import ExitStack

import numpy as np

import concourse.bacc as bacc
import concourse.tile as tile
from concourse import mybir
from concourse.bass_utils import run_bass_kernel_spmd

F32 = mybir.dt.float32
OP = mybir.AluOpType
AF = mybir.ActivationFunctionType

# Problem sizes (hardcoded per harness contract).
B, S, D, C = 64, 8192, 256, 100
NCORES = 8
BPC = B // NCORES          # batches per core
CHUNK = 128                # support rows per tile (partition dim)
NCHUNK = S // CHUNK        # 64

# Constant softmax shift: exp(SHIFT - dist). Exact math (softmax is
# shift-invariant); dist concentrates near sqrt(2*D) ~ 22.6.
SHIFT = 22.0

# Minimax linear seed for sqrt(v) on v in [250, 900] (dist^2 range with huge
# margin), refined by two Newton-Raphson steps -> rel err ~1e-7.
FIT_B = 0.0218287
FIT_A = 10.9031


def _build_nc(bpc=BPC, s=S):
    nchunk = s // CHUNK
    nc = bacc.Bacc(None)
    X = nc.declare_dram_parameter("x", [bpc, s, D], F32, isOutput=False)
    L = nc.declare_dram_parameter("l", [bpc, s, C], F32, isOutput=False)
    M2Q = nc.declare_dram_parameter("m2q", [bpc, 128, D], F32, isOutput=False)
    QQ = nc.declare_dram_parameter("qq", [bpc, 128, 1], F32, isOutput=False)
    OUT = nc.declare_dram_parameter("out", [bpc, C], F32, isOutput=True)

    with tile.TileContext(nc) as tc, ExitStack() as ctx:
        xpool = ctx.enter_context(tc.tile_pool(name="xpool", bufs=4))
        lpool = ctx.enter_context(tc.tile_pool(name="lpool", bufs=4))
        dscp = ctx.enter_context(tc.tile_pool(name="dscp", bufs=2))
        ascp = ctx.enter_context(tc.tile_pool(name="ascp", bufs=2))
        qpool = ctx.enter_context(tc.tile_pool(name="qpool", bufs=2))
        stats = ctx.enter_context(tc.tile_pool(name="stats", bufs=2))
        outp = ctx.enter_context(tc.tile_pool(name="outp", bufs=2))
        cons = ctx.enter_context(tc.tile_pool(name="cons", bufs=1))
        psum = ctx.enter_context(tc.tile_pool(name="psum", bufs=2, space="PSUM"))

        ones = cons.tile([128, 1], F32)
        nc.vector.memset(ones[:], 1.0)
        shiftt = cons.tile([128, 1], F32)
        nc.vector.memset(shiftt[:], SHIFT)

        for b in range(bpc):
            m2q = qpool.tile([128, D], F32, tag="m2q")
            nc.sync.dma_start(m2q[:], M2Q[b])
            qq = qpool.tile([128, 1], F32, tag="qq")
            nc.sync.dma_start(qq[:], QQ[b])

            m2qx = stats.tile([128, nchunk], F32, tag="m2qx")
            sq = stats.tile([128, nchunk], F32, tag="sq")

            # Phase A: stream X, per-row partial distance stats.
            for j in range(nchunk):
                xt = xpool.tile([CHUNK, D], F32, tag="xt")
                nc.sync.dma_start(xt[:], X[b, j * CHUNK:(j + 1) * CHUNK, :])
                sc1 = dscp.tile([CHUNK, D], F32, tag="sc1")
                nc.vector.scalar_tensor_tensor(
                    out=sc1[:],
                    in0=xt[:],
                    scalar=1.0,
                    in1=m2q[:],
                    op0=OP.bypass,
                    op1=OP.mult,
                    accum_out=m2qx[:, j:j + 1],
                )
                sc2 = ascp.tile([CHUNK, D], F32, tag="sc2")
                nc.scalar.activation(
                    out=sc2[:],
                    in_=xt[:],
                    func=AF.Square,
                    accum_out=sq[:, j:j + 1],
                )

            # Phase B: v = dist^2 = (sq + qq) + m2qx; dist via NR sqrt; p = exp(SHIFT - dist)
            v = stats.tile([128, nchunk], F32, tag="v")
            nc.vector.scalar_tensor_tensor(
                out=v[:], in0=sq[:], scalar=qq[:], in1=m2qx[:],
                op0=OP.add, op1=OP.add,
            )
            y0 = stats.tile([128, nchunk], F32, tag="y0")
            nc.vector.tensor_scalar(
                out=y0[:], in0=v[:], scalar1=FIT_B, scalar2=FIT_A,
                op0=OP.mult, op1=OP.add,
            )
            ycur = y0
            for it in range(2):
                r = stats.tile([128, nchunk], F32, tag=f"r{it}")
                nc.vector.reciprocal(r[:], ycur[:])
                t = stats.tile([128, nchunk], F32, tag=f"t{it}")
                nc.vector.tensor_mul(t[:], v[:], r[:])
                u = stats.tile([128, nchunk], F32, tag=f"u{it}")
                nc.vector.tensor_add(u[:], ycur[:], t[:])
                ynext = stats.tile([128, nchunk], F32, tag=f"y{it + 1}")
                nc.vector.tensor_scalar(
                    out=ynext[:], in0=u[:], scalar1=0.5, scalar2=None, op0=OP.mult,
                )
                ycur = ynext

            p = stats.tile([128, nchunk], F32, tag="p")
            nc.scalar.activation(
                out=p[:], in_=ycur[:], func=AF.Exp, scale=-1.0, bias=shiftt[:],
            )

            # Normalizer Z = sum(p) (free-dim reduce, then partition reduce by matmul).
            zc = stats.tile([128, 1], F32, tag="zc")
            nc.vector.reduce_sum(zc[:], p[:], axis=mybir.AxisListType.X)
            zp = psum.tile([1, 1], F32, tag="zp")
            nc.tensor.matmul(zp[:], ones[:], zc[:], start=True, stop=True)
            rz = stats.tile([1, 1], F32, tag="rz")
            nc.vector.reciprocal(rz[:], zp[:])

            # Phase C: out = (p @ labels) / Z, accumulated over chunks in PSUM.
            acc = psum.tile([1, C], F32, tag="acc")
            for j in range(nchunk):
                lt = lpool.tile([CHUNK, C], F32, tag="lt")
                nc.sync.dma_start(lt[:], L[b, j * CHUNK:(j + 1) * CHUNK, :])
                nc.tensor.matmul(
                    acc[:], p[:, j:j + 1], lt[:],
                    start=(j == 0), stop=(j == nchunk - 1),
                )

            ob = outp.tile([1, C], F32, tag="ob")
            nc.vector.tensor_scalar(
                out=ob[:], in0=acc[:], scalar1=rz[:], scalar2=None, op0=OP.mult,
            )
            nc.sync.dma_start(OUT[b:b + 1, :], ob[:])

    nc.finalize()
    return nc


_NC_CACHE = []
LAST_RESULT = None


def kernel(**inputs) -> np.ndarray:
    global LAST_RESULT
    q = np.ascontiguousarray(np.asarray(inputs["query_feats"], dtype=np.float32))
    X = np.ascontiguousarray(np.asarray(inputs["support_feats"], dtype=np.float32))
    L = np.ascontiguousarray(np.asarray(inputs["support_labels"], dtype=np.float32))
    assert q.shape == (B, D) and X.shape == (B, S, D) and L.shape == (B, S, C)

    if not _NC_CACHE:
        _NC_CACHE.append(_build_nc())
    nc = _NC_CACHE[0]

    in_maps = []
    for c in range(NCORES):
        sl = slice(c * BPC, (c + 1) * BPC)
        qb = q[sl]  # (BPC, D)
        m2q = np.ascontiguousarray(
            np.broadcast_to((-2.0 * qb)[:, None, :], (BPC, 128, D)), dtype=np.float32
        )
        qqv = (qb * qb).sum(axis=-1).astype(np.float32)  # (BPC,)
        qq = np.ascontiguousarray(
            np.broadcast_to(qqv[:, None, None], (BPC, 128, 1)), dtype=np.float32
        )
        in_maps.append({"x": X[sl], "l": L[sl], "m2q": m2q, "qq": qq})

    res = run_bass_kernel_spmd(nc, in_maps, list(range(NCORES)))
    LAST_RESULT = res
    out = np.concatenate([res.results[c]["out"] for c in range(NCORES)], axis=0)
    return out.astype(np.float32)


# revision 12
# speedup vs baseline: 2.3298x; 2.3298x over previous
"""Nadaraya-Watson head (retrieval kNN) Trainium2 Bass kernel.

reference:
    dist = ||q - x||_2 over d            (b, s)
    probs = softmax(-dist, axis=s)       (b, s)
    out = probs @ labels                 (b, c)

Strategy (8 NeuronCores, batch-parallel, 8 batches per core):
  dist^2 = sum_d x^2 - 2 q.x + ||q||^2 computed in natural [s=partition,
  d=free] layout so no transposes are needed:
    - DVE: tensor_tensor_reduce(X * (-2q_bcast)) -> per-row -2q.x
    - ACT: activation(Square, accum_out=...)     -> per-row sum x^2
  dist = sqrt via linear seed + 2 Newton-Raphson steps on DVE (avoids the
  sqrt ACT table set; Square/Exp share the exp_and_others set -> one table
  load total).
  Softmax shift is a constant (exact math; values are concentrated around
  dist ~ 22.6 so exp stays in range without a max pass).
  Label reduction: PE matmul with probs column [128,1] stationary and the
  label tile [128,100] moving, accumulated over 64 chunks in PSUM [1,100].
  Normalizer Z via reduce + ones-matmul; final scale by 1/Z on DVE.

The -2q broadcast tile and ||q||^2 are tiny (per-batch) and precomputed on
the host.
"""

from contextlib import ExitStack

import numpy as np

import concourse.bacc as bacc
import concourse.tile as tile
from concourse import mybir
from concourse.bass_utils import run_bass_kernel_spmd

F32 = mybir.dt.float32
OP = mybir.AluOpType
AF = mybir.ActivationFunctionType

# Problem sizes (hardcoded per harness contract).
B, S, D, C = 64, 8192, 256, 100
NCORES = 8
BPC = B // NCORES          # batches per core
CHUNK = 128                # support rows per tile (partition dim)
NCHUNK = S // CHUNK        # 64

# Constant softmax shift: exp(SHIFT - dist). Exact math (softmax is
# shift-invariant); dist concentrates near sqrt(2*D) ~ 22.6.
SHIFT = 22.0

# Minimax linear seed for sqrt(v) on v in [250, 900] (dist^2 range with huge
# margin), refined by two Newton-Raphson steps -> rel err ~1e-7.
FIT_B = 0.0218287
FIT_A = 10.9031


def _build_nc(bpc=BPC, s=S, kpack=8):
    """kpack = consecutive support rows packed per SBUF partition. Raises the
    DMA per-partition line size (kpack*1KB for X, kpack*400B for L) to reach
    full HBM bandwidth; compute runs on [128, D]/[128, C] sub-slices."""
    nchunk = s // CHUNK           # total 128-row score columns
    nblk = s // (CHUNK * kpack)   # DMA tiles per batch
    nc = bacc.Bacc(None)
    X = nc.declare_dram_parameter("x", [bpc, s, D], F32, isOutput=False)
    L = nc.declare_dram_parameter("l", [bpc, s, C], F32, isOutput=False)
    M2Q = nc.declare_dram_parameter("m2q", [bpc, 128, D], F32, isOutput=False)
    QQ = nc.declare_dram_parameter("qq", [bpc, 128, 1], F32, isOutput=False)
    OUT = nc.declare_dram_parameter("out", [bpc, C], F32, isOutput=True)

    with tile.TileContext(nc) as tc, ExitStack() as ctx:
        xpool = ctx.enter_context(tc.tile_pool(name="xpool", bufs=4))
        lpool = ctx.enter_context(tc.tile_pool(name="lpool", bufs=4))
        dscp = ctx.enter_context(tc.tile_pool(name="dscp", bufs=2))
        ascp = ctx.enter_context(tc.tile_pool(name="ascp", bufs=2))
        qpool = ctx.enter_context(tc.tile_pool(name="qpool", bufs=2))
        stats = ctx.enter_context(tc.tile_pool(name="stats", bufs=2))
        outp = ctx.enter_context(tc.tile_pool(name="outp", bufs=2))
        cons = ctx.enter_context(tc.tile_pool(name="cons", bufs=1))
        psum = ctx.enter_context(tc.tile_pool(name="psum", bufs=2, space="PSUM"))

        ones = cons.tile([128, 1], F32)
        nc.vector.memset(ones[:], 1.0)
        shiftt = cons.tile([128, 1], F32)
        nc.vector.memset(shiftt[:], SHIFT)

        for b in range(bpc):
            # [s, D] -> [nblk, 128, kpack*D]: partition p of block j holds rows
            # 128*kpack*j + kpack*p .. +kpack-1 (contiguous kpack*D*4 bytes).
            Xb = X[b].rearrange("(n p k) d -> n p (k d)", p=CHUNK, k=kpack)
            Lb = L[b].rearrange("(n p k) c -> n p (k c)", p=CHUNK, k=kpack)

            m2q = qpool.tile([128, D], F32, tag="m2q")
            nc.sync.dma_start(m2q[:], M2Q[b])
            qq = qpool.tile([128, 1], F32, tag="qq")
            nc.sync.dma_start(qq[:], QQ[b])

            m2qx = stats.tile([128, nchunk], F32, tag="m2qx")
            sq = stats.tile([128, nchunk], F32, tag="sq")

            # Phase A: stream X, per-row partial distance stats.
            # Score column j*kpack + a <-> support row 128*kpack*j + kpack*p + a.
            for j in range(nblk):
                xt = xpool.tile([CHUNK, kpack * D], F32, tag="xt")
                nc.sync.dma_start(xt[:], Xb[j])
                for a in range(kpack):
                    col = j * kpack + a
                    xs = xt[:, a * D:(a + 1) * D]
                    sc1 = dscp.tile([CHUNK, D], F32, tag="sc1")
                    nc.vector.scalar_tensor_tensor(
                        out=sc1[:],
                        in0=xs,
                        scalar=1.0,
                        in1=m2q[:],
                        op0=OP.bypass,
                        op1=OP.mult,
                        accum_out=m2qx[:, col:col + 1],
                    )
                    sc2 = ascp.tile([CHUNK, D], F32, tag="sc2")
                    nc.scalar.activation(
                        out=sc2[:],
                        in_=xs,
                        func=AF.Square,
                        accum_out=sq[:, col:col + 1],
                    )

            # Phase B: v = dist^2 = (sq + qq) + m2qx; dist via NR sqrt; p = exp(SHIFT - dist)
            v = stats.tile([128, nchunk], F32, tag="v")
            nc.vector.scalar_tensor_tensor(
                out=v[:], in0=sq[:], scalar=qq[:], in1=m2qx[:],
                op0=OP.add, op1=OP.add,
            )
            y0 = stats.tile([128, nchunk], F32, tag="y0")
            nc.vector.tensor_scalar(
                out=y0[:], in0=v[:], scalar1=FIT_B, scalar2=FIT_A,
                op0=OP.mult, op1=OP.add,
            )
            ycur = y0
            for it in range(2):
                r = stats.tile([128, nchunk], F32, tag=f"r{it}")
                nc.vector.reciprocal(r[:], ycur[:])
                t = stats.tile([128, nchunk], F32, tag=f"t{it}")
                nc.vector.tensor_mul(t[:], v[:], r[:])
                u = stats.tile([128, nchunk], F32, tag=f"u{it}")
                nc.vector.tensor_add(u[:], ycur[:], t[:])
                ynext = stats.tile([128, nchunk], F32, tag=f"y{it + 1}")
                nc.vector.tensor_scalar(
                    out=ynext[:], in0=u[:], scalar1=0.5, scalar2=None, op0=OP.mult,
                )
                ycur = ynext

            p = stats.tile([128, nchunk], F32, tag="p")
            nc.scalar.activation(
                out=p[:], in_=ycur[:], func=AF.Exp, scale=-1.0, bias=shiftt[:],
            )

            # Normalizer Z = sum(p) (free-dim reduce, then partition reduce by matmul).
            zc = stats.tile([128, 1], F32, tag="zc")
            nc.vector.reduce_sum(zc[:], p[:], axis=mybir.AxisListType.X)
            zp = psum.tile([1, 1], F32, tag="zp")
            nc.tensor.matmul(zp[:], ones[:], zc[:], start=True, stop=True)
            rz = stats.tile([1, 1], F32, tag="rz")
            nc.vector.reciprocal(rz[:], zp[:])

            # Phase C: out = (p @ labels) / Z, accumulated over chunks in PSUM.
            acc = psum.tile([1, C], F32, tag="acc")
            for j in range(nblk):
                lt = lpool.tile([CHUNK, kpack * C], F32, tag="lt")
                nc.sync.dma_start(lt[:], Lb[j])
                for a in range(kpack):
                    col = j * kpack + a
                    nc.tensor.matmul(
                        acc[:], p[:, col:col + 1], lt[:, a * C:(a + 1) * C],
                        start=(col == 0), stop=(col == nchunk - 1),
                    )

            ob = outp.tile([1, C], F32, tag="ob")
            nc.vector.tensor_scalar(
                out=ob[:], in0=acc[:], scalar1=rz[:], scalar2=None, op0=OP.mult,
            )
            nc.sync.dma_start(OUT[b:b + 1, :], ob[:])

    nc.finalize()
    return nc


_NC_CACHE = []
LAST_RESULT = None


def kernel(**inputs) -> np.ndarray:
    global LAST_RESULT
    q = np.ascontiguousarray(np.asarray(inputs["query_feats"], dtype=np.float32))
    X = np.ascontiguousarray(np.asarray(inputs["support_feats"], dtype=np.float32))
    L = np.ascontiguousarray(np.asarray(inputs["support_labels"], dtype=np.float32))
    assert q.shape == (B, D) and X.shape == (B, S, D) and L.shape == (B, S, C)

    if not _NC_CACHE:
        _NC_CACHE.append(_build_nc())
    nc = _NC_CACHE[0]

    in_maps = []
    for c in range(NCORES):
        sl = slice(c * BPC, (c + 1) * BPC)
        qb = q[sl]  # (BPC, D)
        m2q = np.ascontiguousarray(
            np.broadcast_to((-2.0 * qb)[:, None, :], (BPC, 128, D)), dtype=np.float32
        )
        qqv = (qb * qb).sum(axis=-1).astype(np.float32)  # (BPC,)
        qq = np.ascontiguousarray(
            np.broadcast_to(qqv[:, None, None], (BPC, 128, 1)), dtype=np.float32
        )
        in_maps.append({"x": X[sl], "l": L[sl], "m2q": m2q, "qq": qq})

    res = run_bass_kernel_spmd(nc, in_maps, list(range(NCORES)))
    LAST_RESULT = res
    out = np.concatenate([res.results[c]["out"] for c in range(NCORES)], axis=0)
    return out.astype(np.float32)


# revision 13
# speedup vs baseline: 2.9404x; 1.2621x over previous
"""Nadaraya-Watson head (retrieval kNN) Trainium2 Bass kernel.

reference:
    dist = ||q - x||_2 over d            (b, s)
    probs = softmax(-dist, axis=s)       (b, s)
    out = probs @ labels                 (b, c)

Strategy (8 NeuronCores, batch-parallel, 8 batches per core):
  dist^2 = sum_d x^2 - 2 q.x + ||q||^2 computed in natural [s=partition,
  d=free] layout so no transposes are needed:
    - DVE scalar_tensor_tensor(X * (-2q_bcast), accum_out) -> per-row -2q.x
    - per-row sum x^2 split between ACT activation(Square, accum_out) and
      DVE scalar_tensor_tensor(X*X, accum_out) to balance the two engines
  dist = sqrt via linear seed + 2 Newton-Raphson steps on DVE (avoids the
  sqrt ACT table set; Square/Exp share the exp_and_others set -> one table
  load total).
  Softmax shift is a constant (exact math; dist concentrates near 22.6 so
  exp stays in range without a max pass).
  Label reduction: PE matmul with the bf16 probs column [128,1] stationary
  and the label tile [128,101] moving (col 100 = host-appended ones column,
  which makes the softmax normalizer Z fall out of the same fp32 PSUM
  accumulation). Final scale by 1/Z on DVE.

  X, L(+ones), -2q are cast to bf16 on the host: halves HBM traffic (the
  memory-bound term), enables DVE 2x packing, and single-pass PE matmuls.
  All reductions/accumulations stay fp32 (PSUM, accum_out, stats math).

  DMA layout: kpack=16 consecutive support rows packed per SBUF partition
  -> 8KB (X) / 3.2KB (L) per-partition DMA lines, needed for full HBM
  bandwidth. Score column j*kpack + a <-> support row
  128*kpack*j + kpack*p + a on partition p; the label matmul consumes the
  matching L sub-slice, so ordering stays consistent.
"""

from contextlib import ExitStack

import ml_dtypes
import numpy as np

import concourse.bacc as bacc
import concourse.tile as tile
from concourse import mybir
from concourse.bass_utils import run_bass_kernel_spmd

F32 = mybir.dt.float32
BF16 = mybir.dt.bfloat16
OP = mybir.AluOpType
AF = mybir.ActivationFunctionType

# Problem sizes (hardcoded per harness contract).
B, S, D, C = 64, 8192, 256, 100
CA = C + 1                 # labels + ones column
NCORES = 8
BPC = B // NCORES          # batches per core
CHUNK = 128                # support rows per tile (partition dim)

# Constant softmax shift: exp(SHIFT - dist). Exact math (softmax is
# shift-invariant); dist concentrates near sqrt(2*D) ~ 22.6.
SHIFT = 22.0

# Minimax linear seed for sqrt(v) on v in [250, 900] (dist^2 range with huge
# margin), refined by two Newton-Raphson steps -> rel err ~1e-7.
FIT_B = 0.0218287
FIT_A = 10.9031


def _build_nc(bpc=BPC, s=S, kpack=16, dve_sq_n=5):
    """kpack: consecutive support rows per SBUF partition (DMA line size).
    dve_sq_n: of every kpack sum-x^2 ops, this many run on DVE (rest ACT)."""
    nchunk = s // CHUNK           # score columns per batch
    nblk = s // (CHUNK * kpack)   # DMA tiles per batch
    nc = bacc.Bacc(None)
    X = nc.declare_dram_parameter("x", [bpc, s, D], BF16, isOutput=False)
    L = nc.declare_dram_parameter("l", [bpc, s, CA], BF16, isOutput=False)
    M2Q = nc.declare_dram_parameter("m2q", [bpc, 128, D], BF16, isOutput=False)
    QQ = nc.declare_dram_parameter("qq", [bpc, 128, 1], F32, isOutput=False)
    OUT = nc.declare_dram_parameter("out", [bpc, C], F32, isOutput=True)

    with tile.TileContext(nc) as tc, ExitStack() as ctx:
        xpool = ctx.enter_context(tc.tile_pool(name="xpool", bufs=3))
        lpool = ctx.enter_context(tc.tile_pool(name="lpool", bufs=3))
        dscp = ctx.enter_context(tc.tile_pool(name="dscp", bufs=2))
        ascp = ctx.enter_context(tc.tile_pool(name="ascp", bufs=2))
        qpool = ctx.enter_context(tc.tile_pool(name="qpool", bufs=2))
        stats = ctx.enter_context(tc.tile_pool(name="stats", bufs=2))
        outp = ctx.enter_context(tc.tile_pool(name="outp", bufs=2))
        cons = ctx.enter_context(tc.tile_pool(name="cons", bufs=1))
        psum = ctx.enter_context(tc.tile_pool(name="psum", bufs=2, space="PSUM"))

        shiftt = cons.tile([128, 1], F32)
        nc.vector.memset(shiftt[:], SHIFT)

        for b in range(bpc):
            Xb = X[b].rearrange("(n p k) d -> n p (k d)", p=CHUNK, k=kpack)
            Lb = L[b].rearrange("(n p k) c -> n p (k c)", p=CHUNK, k=kpack)

            m2q = qpool.tile([128, D], BF16, tag="m2q")
            nc.sync.dma_start(m2q[:], M2Q[b])
            qq = qpool.tile([128, 1], F32, tag="qq")
            nc.sync.dma_start(qq[:], QQ[b])

            m2qx = stats.tile([128, nchunk], F32, tag="m2qx")
            sq = stats.tile([128, nchunk], F32, tag="sq")

            # Phase A: stream X, per-row partial distance stats.
            for j in range(nblk):
                xt = xpool.tile([CHUNK, kpack * D], BF16, tag="xt")
                nc.sync.dma_start(xt[:], Xb[j])
                for a in range(kpack):
                    col = j * kpack + a
                    xs = xt[:, a * D:(a + 1) * D]
                    sc1 = dscp.tile([CHUNK, D], BF16, tag="sc1")
                    nc.vector.scalar_tensor_tensor(
                        out=sc1[:], in0=xs, scalar=1.0, in1=m2q[:],
                        op0=OP.bypass, op1=OP.mult,
                        accum_out=m2qx[:, col:col + 1],
                    )
                    if a < dve_sq_n:
                        sc3 = dscp.tile([CHUNK, D], BF16, tag="sc3")
                        nc.vector.scalar_tensor_tensor(
                            out=sc3[:], in0=xs, scalar=1.0, in1=xs,
                            op0=OP.bypass, op1=OP.mult,
                            accum_out=sq[:, col:col + 1],
                        )
                    else:
                        sc2 = ascp.tile([CHUNK, D], BF16, tag="sc2")
                        nc.scalar.activation(
                            out=sc2[:], in_=xs, func=AF.Square,
                            accum_out=sq[:, col:col + 1],
                        )

            # Phase B: v = dist^2 = (sq + qq) + m2qx; dist via NR sqrt;
            # p = exp(SHIFT - dist) downcast to bf16 for the PE matmuls.
            v = stats.tile([128, nchunk], F32, tag="v")
            nc.vector.scalar_tensor_tensor(
                out=v[:], in0=sq[:], scalar=qq[:], in1=m2qx[:],
                op0=OP.add, op1=OP.add,
            )
            y0 = stats.tile([128, nchunk], F32, tag="y0")
            nc.vector.tensor_scalar(
                out=y0[:], in0=v[:], scalar1=FIT_B, scalar2=FIT_A,
                op0=OP.mult, op1=OP.add,
            )
            ycur = y0
            for it in range(2):
                r = stats.tile([128, nchunk], F32, tag=f"r{it}")
                nc.vector.reciprocal(r[:], ycur[:])
                t = stats.tile([128, nchunk], F32, tag=f"t{it}")
                nc.vector.tensor_mul(t[:], v[:], r[:])
                u = stats.tile([128, nchunk], F32, tag=f"u{it}")
                nc.vector.tensor_add(u[:], ycur[:], t[:])
                ynext = stats.tile([128, nchunk], F32, tag=f"y{it + 1}")
                nc.vector.tensor_scalar(
                    out=ynext[:], in0=u[:], scalar1=0.5, scalar2=None, op0=OP.mult,
                )
                ycur = ynext

            p = stats.tile([128, nchunk], BF16, tag="p")
            nc.scalar.activation(
                out=p[:], in_=ycur[:], func=AF.Exp, scale=-1.0, bias=shiftt[:],
            )

            # Phase C: psum[0, 0:100] = sum_s p_s * L[s, :], psum[0, 100] = Z.
            acc = psum.tile([1, CA], F32, tag="acc")
            for j in range(nblk):
                lt = lpool.tile([CHUNK, kpack * CA], BF16, tag="lt")
                nc.sync.dma_start(lt[:], Lb[j])
                for a in range(kpack):
                    col = j * kpack + a
                    nc.tensor.matmul(
                        acc[:], p[:, col:col + 1], lt[:, a * CA:(a + 1) * CA],
                        start=(col == 0), stop=(col == nchunk - 1),
                    )

            rz = stats.tile([1, 1], F32, tag="rz")
            nc.vector.reciprocal(rz[:], acc[0:1, C:C + 1])
            ob = outp.tile([1, C], F32, tag="ob")
            nc.vector.tensor_scalar(
                out=ob[:], in0=acc[0:1, 0:C], scalar1=rz[:], scalar2=None,
                op0=OP.mult,
            )
            nc.sync.dma_start(OUT[b:b + 1, :], ob[:])

    nc.finalize()
    return nc


_NC_CACHE = []
LAST_RESULT = None
BF = ml_dtypes.bfloat16


def _prep_core(q, X, L):
    """Host-side prep for one core's slice: bf16 casts, ones column, -2q
    broadcast, ||q||^2."""
    bpc = q.shape[0]
    s = X.shape[1]
    Laug = np.empty((bpc, s, CA), dtype=BF)
    Laug[:, :, :C] = L
    Laug[:, :, C] = 1.0
    m2q = np.ascontiguousarray(
        np.broadcast_to((-2.0 * q).astype(BF)[:, None, :], (bpc, 128, D))
    )
    qqv = (q.astype(np.float64) ** 2).sum(-1).astype(np.float32)
    qq = np.ascontiguousarray(
        np.broadcast_to(qqv[:, None, None], (bpc, 128, 1)), dtype=np.float32
    )
    return {"x": X.astype(BF), "l": Laug, "m2q": m2q, "qq": qq}


def kernel(**inputs) -> np.ndarray:
    global LAST_RESULT
    q = np.asarray(inputs["query_feats"], dtype=np.float32)
    X = np.asarray(inputs["support_feats"], dtype=np.float32)
    L = np.asarray(inputs["support_labels"], dtype=np.float32)
    assert q.shape == (B, D) and X.shape == (B, S, D) and L.shape == (B, S, C)

    if not _NC_CACHE:
        _NC_CACHE.append(_build_nc())
    nc = _NC_CACHE[0]

    in_maps = []
    for c in range(NCORES):
        sl = slice(c * BPC, (c + 1) * BPC)
        in_maps.append(_prep_core(q[sl], X[sl], L[sl]))

    res = run_bass_kernel_spmd(nc, in_maps, list(range(NCORES)))
    LAST_RESULT = res
    out = np.concatenate([res.results[c]["out"] for c in range(NCORES)], axis=0)
    return out.astype(np.float32)


# revision 14
# speedup vs baseline: 3.4990x; 1.1900x over previous
"""Nadaraya-Watson head (retrieval kNN) Trainium2 Bass kernel.

reference:
    dist = ||q - x||_2 over d            (b, s)
    probs = softmax(-dist, axis=s)       (b, s)
    out = probs @ labels                 (b, c)

Strategy (8 NeuronCores, batch-parallel, 8 batches per core):
  All big operands are bf16 (host-cast): halves HBM traffic (the
  memory-bound term), enables DVE 2x packing, single-pass PE matmuls.
  Reductions/accumulations stay fp32 (PSUM, accum_out, stats math).

  dist^2 = sum_d (x - q)^2 computed in natural [s=partition, d=free]
  layout (no transposes):
    - one wide DVE tensor_sub per [128, kpack*D] tile: W = X - q, with q
      read through a stride-0 broadcast AP (2x bf16 mode)
    - per-row sum W^2 via activation(Square, accum_out) on ACT and
      scalar_tensor_tensor(W*W, accum_out) on DVE, split to balance the
      two engines (DVE_SQ_N of every kpack go to DVE)
  dist = sqrt via linear seed + 2 Newton-Raphson steps on DVE (avoids the
  sqrt ACT table set; Square/Exp share the exp_and_others set -> one table
  load total).
  Softmax shift is a constant (exact math; dist concentrates near 22.6 so
  exp stays in range without a max pass).
  Label reduction: PE matmul with the bf16 probs column [128,1] stationary
  and the label tile [128,101] moving (col 100 = host-appended ones column,
  which makes the softmax normalizer Z fall out of the same fp32 PSUM
  accumulation). Final scale by 1/Z on DVE.

  DMA layout: kpack=16 consecutive support rows packed per SBUF partition
  -> 8KB (X) / 3.2KB (L) per-partition DMA lines, needed for full HBM
  bandwidth. Score column j*kpack + a <-> support row
  128*kpack*j + kpack*p + a on partition p; the label matmul consumes the
  matching L sub-slice, so ordering stays consistent.
"""

from contextlib import ExitStack

import ml_dtypes
import numpy as np

import concourse.bacc as bacc
import concourse.tile as tile
from concourse import mybir
from concourse.bass_utils import run_bass_kernel_spmd

F32 = mybir.dt.float32
BF16 = mybir.dt.bfloat16
OP = mybir.AluOpType
AF = mybir.ActivationFunctionType

# Problem sizes (hardcoded per harness contract).
B, S, D, C = 64, 8192, 256, 100
CA = C + 1                 # labels + ones column
NCORES = 8
BPC = B // NCORES          # batches per core
CHUNK = 128                # support rows per tile (partition dim)

# Constant softmax shift: exp(SHIFT - dist). Exact math (softmax is
# shift-invariant); dist concentrates near sqrt(2*D) ~ 22.6.
SHIFT = 22.0

# Minimax linear seed for sqrt(v) on v in [250, 900] (dist^2 range with huge
# margin), refined by two Newton-Raphson steps -> rel err ~1e-7.
FIT_B = 0.0218287
FIT_A = 10.9031


def _build_nc(bpc=BPC, s=S, kpack=16, dve_sq_n=5):
    """kpack: consecutive support rows per SBUF partition (DMA line size).
    dve_sq_n: of every kpack sum-W^2 ops, this many run on DVE (rest ACT)."""
    nchunk = s // CHUNK           # score columns per batch
    nblk = s // (CHUNK * kpack)   # DMA tiles per batch
    nc = bacc.Bacc(None)
    X = nc.declare_dram_parameter("x", [bpc, s, D], BF16, isOutput=False)
    L = nc.declare_dram_parameter("l", [bpc, s, CA], BF16, isOutput=False)
    QR = nc.declare_dram_parameter("qr", [bpc, 128, D], BF16, isOutput=False)
    OUT = nc.declare_dram_parameter("out", [bpc, C], F32, isOutput=True)

    with tile.TileContext(nc) as tc, ExitStack() as ctx:
        xpool = ctx.enter_context(tc.tile_pool(name="xpool", bufs=3))
        lpool = ctx.enter_context(tc.tile_pool(name="lpool", bufs=3))
        wpool = ctx.enter_context(tc.tile_pool(name="wpool", bufs=3))
        dscp = ctx.enter_context(tc.tile_pool(name="dscp", bufs=2))
        ascp = ctx.enter_context(tc.tile_pool(name="ascp", bufs=2))
        qpool = ctx.enter_context(tc.tile_pool(name="qpool", bufs=2))
        stats = ctx.enter_context(tc.tile_pool(name="stats", bufs=2))
        outp = ctx.enter_context(tc.tile_pool(name="outp", bufs=2))
        cons = ctx.enter_context(tc.tile_pool(name="cons", bufs=1))
        psum = ctx.enter_context(tc.tile_pool(name="psum", bufs=2, space="PSUM"))

        shiftt = cons.tile([128, 1], F32)
        nc.vector.memset(shiftt[:], SHIFT)

        for b in range(bpc):
            Xb = X[b].rearrange("(n p k) d -> n p (k d)", p=CHUNK, k=kpack)
            Lb = L[b].rearrange("(n p k) c -> n p (k c)", p=CHUNK, k=kpack)

            qr = qpool.tile([128, D], BF16, tag="qr")
            nc.sync.dma_start(qr[:], QR[b])
            qbc = qr[:].rearrange("p (k d) -> p k d", k=1).to_broadcast(
                (128, kpack, D)
            )

            v = stats.tile([128, nchunk], F32, tag="v")

            # Phase A: stream X; W = X - q; v[:, col] = sum_d W^2 = dist^2.
            for j in range(nblk):
                xt = xpool.tile([CHUNK, kpack * D], BF16, tag="xt")
                nc.sync.dma_start(xt[:], Xb[j])
                wt = wpool.tile([CHUNK, kpack * D], BF16, tag="wt")
                nc.vector.tensor_sub(
                    wt[:].rearrange("p (k d) -> p k d", k=kpack), xt[:].rearrange("p (k d) -> p k d", k=kpack), qbc
                )
                for a in range(kpack):
                    col = j * kpack + a
                    ws = wt[:, a * D:(a + 1) * D]
                    if a < dve_sq_n:
                        sc1 = dscp.tile([CHUNK, D], BF16, tag="sc1")
                        nc.vector.scalar_tensor_tensor(
                            out=sc1[:], in0=ws, scalar=1.0, in1=ws,
                            op0=OP.bypass, op1=OP.mult,
                            accum_out=v[:, col:col + 1],
                        )
                    else:
                        sc2 = ascp.tile([CHUNK, D], BF16, tag="sc2")
                        nc.scalar.activation(
                            out=sc2[:], in_=ws, func=AF.Square,
                            accum_out=v[:, col:col + 1],
                        )

            # Phase B: dist via NR sqrt; p = exp(SHIFT - dist) in bf16.
            y0 = stats.tile([128, nchunk], F32, tag="y0")
            nc.vector.tensor_scalar(
                out=y0[:], in0=v[:], scalar1=FIT_B, scalar2=FIT_A,
                op0=OP.mult, op1=OP.add,
            )
            ycur = y0
            for it in range(2):
                r = stats.tile([128, nchunk], F32, tag=f"r{it}")
                nc.vector.reciprocal(r[:], ycur[:])
                t = stats.tile([128, nchunk], F32, tag=f"t{it}")
                nc.vector.tensor_mul(t[:], v[:], r[:])
                u = stats.tile([128, nchunk], F32, tag=f"u{it}")
                nc.vector.tensor_add(u[:], ycur[:], t[:])
                ynext = stats.tile([128, nchunk], F32, tag=f"y{it + 1}")
                nc.vector.tensor_scalar(
                    out=ynext[:], in0=u[:], scalar1=0.5, scalar2=None, op0=OP.mult,
                )
                ycur = ynext

            p = stats.tile([128, nchunk], BF16, tag="p")
            nc.scalar.activation(
                out=p[:], in_=ycur[:], func=AF.Exp, scale=-1.0, bias=shiftt[:],
            )

            # Phase C: psum[0, 0:100] = sum_s p_s * L[s, :], psum[0, 100] = Z.
            acc = psum.tile([1, CA], F32, tag="acc")
            for j in range(nblk):
                lt = lpool.tile([CHUNK, kpack * CA], BF16, tag="lt")
                nc.sync.dma_start(lt[:], Lb[j])
                for a in range(kpack):
                    col = j * kpack + a
                    nc.tensor.matmul(
                        acc[:], p[:, col:col + 1], lt[:, a * CA:(a + 1) * CA],
                        start=(col == 0), stop=(col == nchunk - 1),
                    )

            rz = stats.tile([1, 1], F32, tag="rz")
            nc.vector.reciprocal(rz[:], acc[0:1, C:C + 1])
            ob = outp.tile([1, C], F32, tag="ob")
            nc.vector.tensor_scalar(
                out=ob[:], in0=acc[0:1, 0:C], scalar1=rz[:], scalar2=None,
                op0=OP.mult,
            )
            nc.sync.dma_start(OUT[b:b + 1, :], ob[:])

    nc.finalize()
    return nc


_NC_CACHE = []
LAST_RESULT = None
BF = ml_dtypes.bfloat16


def _prep_core(q, X, L):
    """Host-side prep for one core's slice: bf16 casts, ones column on L,
    q broadcast."""
    bpc = q.shape[0]
    s = X.shape[1]
    Laug = np.empty((bpc, s, CA), dtype=BF)
    Laug[:, :, :C] = L
    Laug[:, :, C] = 1.0
    qr = np.ascontiguousarray(
        np.broadcast_to(q.astype(BF)[:, None, :], (bpc, 128, D))
    )
    return {"x": X.astype(BF), "l": Laug, "qr": qr}


def kernel(**inputs) -> np.ndarray:
    global LAST_RESULT
    q = np.asarray(inputs["query_feats"], dtype=np.float32)
    X = np.asarray(inputs["support_feats"], dtype=np.float32)
    L = np.asarray(inputs["support_labels"], dtype=np.float32)
    assert q.shape == (B, D) and X.shape == (B, S, D) and L.shape == (B, S, C)

    if not _NC_CACHE:
        _NC_CACHE.append(_build_nc())
    nc = _NC_CACHE[0]

    in_maps = []
    for c in range(NCORES):
        sl = slice(c * BPC, (c + 1) * BPC)
        in_maps.append(_prep_core(q[sl], X[sl], L[sl]))

    res = run_bass_kernel_spmd(nc, in_maps, list(range(NCORES)))
    LAST_RESULT = res
    out = np.concatenate([res.results[c]["out"] for c in range(NCORES)], axis=0)
    return out.astype(np.float32)


# revision 15
# speedup vs baseline: 3.9537x; 1.1300x over previous
"""Nadaraya-Watson head (retrieval kNN) Trainium2 Bass kernel.

reference:
    dist = ||q - x||_2 over d            (b, s)
    probs = softmax(-dist, axis=s)       (b, s)
    out = probs @ labels                 (b, c)

Strategy (8 NeuronCores, batch-parallel, 8 batches per core):
  All big operands are bf16 (host-cast): halves HBM traffic (the
  memory-bound term), enables DVE 2x packing, single-pass PE matmuls.
  Reductions/accumulations stay fp32 (PSUM, accum_out, stats math).

  dist^2 = sum_d (x - q)^2 computed in natural [s=partition, d=free]
  layout (no transposes):
    - one wide DVE tensor_sub per [128, kpack*D] tile: W = X - q, with q
      read through a stride-0 broadcast AP (2x bf16 mode)
    - per-row sum W^2 via activation(Square, accum_out) on ACT and
      scalar_tensor_tensor(W*W, accum_out) on DVE, split to balance the
      two engines (DVE_SQ_N of every kpack go to DVE)
  dist = sqrt via linear seed + 2 Newton-Raphson steps on DVE (avoids the
  sqrt ACT table set; Square/Exp share the exp_and_others set -> one table
  load total).
  Softmax shift is a constant (exact math; dist concentrates near 22.6 so
  exp stays in range without a max pass).
  Label reduction: PE matmul with the bf16 probs column [128,1] stationary
  and the label tile [128,101] moving (col 100 = host-appended ones column,
  which makes the softmax normalizer Z fall out of the same fp32 PSUM
  accumulation). Final scale by 1/Z on DVE.

  DMA layout: kpack=16 consecutive support rows packed per SBUF partition
  -> 8KB (X) / 3.2KB (L) per-partition DMA lines, needed for full HBM
  bandwidth. Score column j*kpack + a <-> support row
  128*kpack*j + kpack*p + a on partition p; the label matmul consumes the
  matching L sub-slice, so ordering stays consistent.
"""

from contextlib import ExitStack

import ml_dtypes
import numpy as np

import concourse.bacc as bacc
import concourse.tile as tile
from concourse import mybir
from concourse.bass_utils import run_bass_kernel_spmd

F32 = mybir.dt.float32
BF16 = mybir.dt.bfloat16
OP = mybir.AluOpType
AF = mybir.ActivationFunctionType

# Problem sizes (hardcoded per harness contract).
B, S, D, C = 64, 8192, 256, 100
CA = C + 1                 # labels + ones column
NCORES = 8
BPC = B // NCORES          # batches per core
CHUNK = 128                # support rows per tile (partition dim)

# Constant softmax shift: exp(SHIFT - dist). Exact math (softmax is
# shift-invariant); dist concentrates near sqrt(2*D) ~ 22.6.
SHIFT = 22.0

# Minimax linear seed for sqrt(v) on v in [250, 900] (dist^2 range with huge
# margin), refined by two Newton-Raphson steps -> rel err ~1e-7.
FIT_B = 0.0218287
FIT_A = 10.9031


def _build_nc(bpc=BPC, s=S, kpack=32, act_accum_n=8):
    """kpack: consecutive support rows per SBUF partition (DMA line size).
    Of every kpack rows, the first act_accum_n get ACT accum-squares; the
    rest go through ACT wide-square -> DVE pair-fold -> DVE 3D reduce."""
    nchunk = s // CHUNK           # score columns per batch
    nblk = s // (CHUNK * kpack)   # DMA tiles per batch
    na = act_accum_n
    nd = kpack - na               # rows per tile on the fold+reduce path
    nc = bacc.Bacc(None)
    X = nc.declare_dram_parameter("x", [bpc, s, D], BF16, isOutput=False)
    L = nc.declare_dram_parameter("l", [bpc, s, CA], BF16, isOutput=False)
    QR = nc.declare_dram_parameter("qr", [bpc, 128, D], BF16, isOutput=False)
    OUT = nc.declare_dram_parameter("out", [bpc, C], F32, isOutput=True)

    with tile.TileContext(nc) as tc, ExitStack() as ctx:
        xpool = ctx.enter_context(tc.tile_pool(name="xpool", bufs=3))
        lpool = ctx.enter_context(tc.tile_pool(name="lpool", bufs=3))
        wpool = ctx.enter_context(tc.tile_pool(name="wpool", bufs=3))
        w2pool = ctx.enter_context(tc.tile_pool(name="w2pool", bufs=2))
        fpool = ctx.enter_context(tc.tile_pool(name="fpool", bufs=2))
        ascp = ctx.enter_context(tc.tile_pool(name="ascp", bufs=2))
        qpool = ctx.enter_context(tc.tile_pool(name="qpool", bufs=2))
        stats = ctx.enter_context(tc.tile_pool(name="stats", bufs=2))
        outp = ctx.enter_context(tc.tile_pool(name="outp", bufs=2))
        cons = ctx.enter_context(tc.tile_pool(name="cons", bufs=1))
        psum = ctx.enter_context(tc.tile_pool(name="psum", bufs=2, space="PSUM"))

        shiftt = cons.tile([128, 1], F32)
        nc.vector.memset(shiftt[:], SHIFT)

        for b in range(bpc):
            Xb = X[b].rearrange("(n p k) d -> n p (k d)", p=CHUNK, k=kpack)
            Lb = L[b].rearrange("(n p k) c -> n p (k c)", p=CHUNK, k=kpack)

            qr = qpool.tile([128, D], BF16, tag="qr")
            nc.sync.dma_start(qr[:], QR[b])
            qbc = qr[:].rearrange("p (k d) -> p k d", k=1).to_broadcast(
                (128, kpack, D)
            )

            v = stats.tile([128, nchunk], F32, tag="v")

            # Phase A: stream X; W = X - q; v[:, col] = sum_d W^2 = dist^2.
            for j in range(nblk):
                xt = xpool.tile([CHUNK, kpack * D], BF16, tag="xt")
                nc.sync.dma_start(xt[:], Xb[j])
                wt = wpool.tile([CHUNK, kpack * D], BF16, tag="wt")
                nc.vector.tensor_sub(
                    wt[:].rearrange("p (k d) -> p k d", k=kpack),
                    xt[:].rearrange("p (k d) -> p k d", k=kpack),
                    qbc,
                )
                # rows [0, na): ACT square with per-row accumulate
                for a in range(na):
                    col = j * kpack + a
                    ws = wt[:, a * D:(a + 1) * D]
                    sc2 = ascp.tile([CHUNK, D], BF16, tag="sc2")
                    nc.scalar.activation(
                        out=sc2[:], in_=ws, func=AF.Square,
                        accum_out=v[:, col:col + 1],
                    )
                # rows [na, kpack): one wide ACT square, then DVE pairwise
                # fold (2x bf16) + one 3D reduce into contiguous v columns.
                w2 = w2pool.tile([CHUNK, nd * D], BF16, tag="w2")
                nc.scalar.activation(
                    out=w2[:], in_=wt[:, na * D:kpack * D], func=AF.Square,
                )
                w23 = w2[:].rearrange("p (k d) -> p k d", k=nd)
                f = fpool.tile([CHUNK, nd * (D // 2)], BF16, tag="f")
                f3 = f[:].rearrange("p (k d) -> p k d", k=nd)
                nc.vector.tensor_add(
                    f3, w23[:, :, 0:D // 2], w23[:, :, D // 2:D]
                )
                nc.vector.tensor_reduce(
                    v[:, j * kpack + na:(j + 1) * kpack], f3,
                    axis=mybir.AxisListType.X, op=OP.add,
                )

            # Phase B: dist via NR sqrt; p = exp(SHIFT - dist) in bf16.
            y0 = stats.tile([128, nchunk], F32, tag="y0")
            nc.vector.tensor_scalar(
                out=y0[:], in0=v[:], scalar1=FIT_B, scalar2=FIT_A,
                op0=OP.mult, op1=OP.add,
            )
            ycur = y0
            for it in range(2):
                r = stats.tile([128, nchunk], F32, tag=f"r{it}")
                nc.vector.reciprocal(r[:], ycur[:])
                t = stats.tile([128, nchunk], F32, tag=f"t{it}")
                nc.vector.tensor_mul(t[:], v[:], r[:])
                u = stats.tile([128, nchunk], F32, tag=f"u{it}")
                nc.vector.tensor_add(u[:], ycur[:], t[:])
                ynext = stats.tile([128, nchunk], F32, tag=f"y{it + 1}")
                nc.vector.tensor_scalar(
                    out=ynext[:], in0=u[:], scalar1=0.5, scalar2=None, op0=OP.mult,
                )
                ycur = ynext

            p = stats.tile([128, nchunk], BF16, tag="p")
            nc.scalar.activation(
                out=p[:], in_=ycur[:], func=AF.Exp, scale=-1.0, bias=shiftt[:],
            )

            # Phase C: psum[0, 0:100] = sum_s p_s * L[s, :], psum[0, 100] = Z.
            acc = psum.tile([1, CA], F32, tag="acc")
            for j in range(nblk):
                lt = lpool.tile([CHUNK, kpack * CA], BF16, tag="lt")
                nc.sync.dma_start(lt[:], Lb[j])
                for a in range(kpack):
                    col = j * kpack + a
                    nc.tensor.matmul(
                        acc[:], p[:, col:col + 1], lt[:, a * CA:(a + 1) * CA],
                        start=(col == 0), stop=(col == nchunk - 1),
                    )

            rz = stats.tile([1, 1], F32, tag="rz")
            nc.vector.reciprocal(rz[:], acc[0:1, C:C + 1])
            ob = outp.tile([1, C], F32, tag="ob")
            nc.vector.tensor_scalar(
                out=ob[:], in0=acc[0:1, 0:C], scalar1=rz[:], scalar2=None,
                op0=OP.mult,
            )
            nc.sync.dma_start(OUT[b:b + 1, :], ob[:])

    nc.finalize()
    return nc


_NC_CACHE = []
LAST_RESULT = None
BF = ml_dtypes.bfloat16


def _prep_core(q, X, L):
    """Host-side prep for one core's slice: bf16 casts, ones column on L,
    q broadcast."""
    bpc = q.shape[0]
    s = X.shape[1]
    Laug = np.empty((bpc, s, CA), dtype=BF)
    Laug[:, :, :C] = L
    Laug[:, :, C] = 1.0
    qr = np.ascontiguousarray(
        np.broadcast_to(q.astype(BF)[:, None, :], (bpc, 128, D))
    )
    return {"x": X.astype(BF), "l": Laug, "qr": qr}


def kernel(**inputs) -> np.ndarray:
    global LAST_RESULT
    q = np.asarray(inputs["query_feats"], dtype=np.float32)
    X = np.asarray(inputs["support_feats"], dtype=np.float32)
    L = np.asarray(inputs["support_labels"], dtype=np.float32)
    assert q.shape == (B, D) and X.shape == (B, S, D) and L.shape == (B, S, C)

    if not _NC_CACHE:
        _NC_CACHE.append(_build_nc())
    nc = _NC_CACHE[0]

    in_maps = []
    for c in range(NCORES):
        sl = slice(c * BPC, (c + 1) * BPC)
        in_maps.append(_prep_core(q[sl], X[sl], L[sl]))

    res = run_bass_kernel_spmd(nc, in_maps, list(range(NCORES)))
    LAST_RESULT = res
    out = np.concatenate([res.results[c]["out"] for c in range(NCORES)], axis=0)
    return out.astype(np.float32)


# revision 17
# speedup vs baseline: 4.1278x; 1.0440x over previous
"""Nadaraya-Watson head (retrieval kNN) Trainium2 Bass kernel.

reference:
    dist = ||q - x||_2 over d            (b, s)
    probs = softmax(-dist, axis=s)       (b, s)
    out = probs @ labels                 (b, c)

Strategy (8 NeuronCores, batch-parallel, 8 batches per core):
  All big operands are bf16 (host-cast): halves HBM traffic (the
  memory-bound term), enables DVE 2x packing, single-pass PE matmuls.
  Reductions/accumulations stay fp32 (PSUM, accum_out, stats math).

  dist^2 = sum_d (x - q)^2 computed in natural [s=partition, d=free]
  layout (no transposes):
    - one wide DVE tensor_sub per [128, kpack*D] tile: W = X - q, with q
      read through a stride-0 broadcast AP (2x bf16 mode)
    - per-row sum W^2 via activation(Square, accum_out) on ACT and
      scalar_tensor_tensor(W*W, accum_out) on DVE, split to balance the
      two engines (DVE_SQ_N of every kpack go to DVE)
  dist = sqrt via linear seed + 2 Newton-Raphson steps on DVE (avoids the
  sqrt ACT table set; Square/Exp share the exp_and_others set -> one table
  load total).
  Softmax shift is a constant (exact math; dist concentrates near 22.6 so
  exp stays in range without a max pass).
  Label reduction: PE matmul with the bf16 probs column [128,1] stationary
  and the label tile [128,101] moving (col 100 = host-appended ones column,
  which makes the softmax normalizer Z fall out of the same fp32 PSUM
  accumulation). Final scale by 1/Z on DVE.

  DMA layout: kpack=16 consecutive support rows packed per SBUF partition
  -> 8KB (X) / 3.2KB (L) per-partition DMA lines, needed for full HBM
  bandwidth. Score column j*kpack + a <-> support row
  128*kpack*j + kpack*p + a on partition p; the label matmul consumes the
  matching L sub-slice, so ordering stays consistent.
"""

from contextlib import ExitStack

import ml_dtypes
import numpy as np

import concourse.bacc as bacc
import concourse.tile as tile
from concourse import mybir
from concourse.bass_utils import run_bass_kernel_spmd

F32 = mybir.dt.float32
BF16 = mybir.dt.bfloat16
OP = mybir.AluOpType
AF = mybir.ActivationFunctionType

# Problem sizes (hardcoded per harness contract).
B, S, D, C = 64, 8192, 256, 100
CA = C + 1                 # labels + ones column
NCORES = 8
BPC = B // NCORES          # batches per core
CHUNK = 128                # support rows per tile (partition dim)

# Constant softmax shift: exp(SHIFT - dist). Exact math (softmax is
# shift-invariant); dist concentrates near sqrt(2*D) ~ 22.6.
SHIFT = 22.0

# Minimax linear seed for sqrt(v) on v in [250, 900] (dist^2 range with huge
# margin), refined by two Newton-Raphson steps -> rel err ~1e-7.
FIT_B = 0.0218287
FIT_A = 10.9031


def _build_nc(bpc=BPC, s=S, kpack=32, act_accum_n=8):
    """kpack: consecutive support rows per SBUF partition (DMA line size).
    Of every kpack rows, the first act_accum_n get ACT accum-squares; the
    rest go through ACT wide-square -> DVE pair-fold -> DVE 3D reduce."""
    nchunk = s // CHUNK           # score columns per batch
    nblk = s // (CHUNK * kpack)   # DMA tiles per batch
    na = act_accum_n
    nd = kpack - na               # rows per tile on the fold+reduce path
    nc = bacc.Bacc(None)
    X = nc.declare_dram_parameter("x", [bpc, s, D], BF16, isOutput=False)
    L = nc.declare_dram_parameter("l", [bpc, s, CA], BF16, isOutput=False)
    QR = nc.declare_dram_parameter("qr", [bpc, 128, D], BF16, isOutput=False)
    OUT = nc.declare_dram_parameter("out", [bpc, C], F32, isOutput=True)

    with tile.TileContext(nc) as tc, ExitStack() as ctx:
        xpool = ctx.enter_context(tc.tile_pool(name="xpool", bufs=4))
        lpool = ctx.enter_context(tc.tile_pool(name="lpool", bufs=4))
        wpool = ctx.enter_context(tc.tile_pool(name="wpool", bufs=2))
        w2pool = ctx.enter_context(tc.tile_pool(name="w2pool", bufs=2))
        fpool = ctx.enter_context(tc.tile_pool(name="fpool", bufs=2))
        ascp = ctx.enter_context(tc.tile_pool(name="ascp", bufs=2))
        qpool = ctx.enter_context(tc.tile_pool(name="qpool", bufs=2))
        stats = ctx.enter_context(tc.tile_pool(name="stats", bufs=2))
        outp = ctx.enter_context(tc.tile_pool(name="outp", bufs=2))
        cons = ctx.enter_context(tc.tile_pool(name="cons", bufs=1))
        psum = ctx.enter_context(tc.tile_pool(name="psum", bufs=2, space="PSUM"))

        shiftt = cons.tile([128, 1], F32)
        nc.vector.memset(shiftt[:], SHIFT)

        for b in range(bpc):
            Xb = X[b].rearrange("(n p k) d -> n p (k d)", p=CHUNK, k=kpack)
            Lb = L[b].rearrange("(n p k) c -> n p (k c)", p=CHUNK, k=kpack)

            qr = qpool.tile([128, D], BF16, tag="qr")
            nc.sync.dma_start(qr[:], QR[b])
            qbc = qr[:].rearrange("p (k d) -> p k d", k=1).to_broadcast(
                (128, kpack, D)
            )

            v = stats.tile([128, nchunk], F32, tag="v")

            # Phase A: stream X; W = X - q; v[:, col] = sum_d W^2 = dist^2.
            for j in range(nblk):
                xt = xpool.tile([CHUNK, kpack * D], BF16, tag="xt")
                nc.sync.dma_start(xt[:], Xb[j])
                wt = wpool.tile([CHUNK, kpack * D], BF16, tag="wt")
                nc.vector.tensor_sub(
                    wt[:].rearrange("p (k d) -> p k d", k=kpack),
                    xt[:].rearrange("p (k d) -> p k d", k=kpack),
                    qbc,
                )
                # rows [0, na): ACT square with per-row accumulate
                for a in range(na):
                    col = j * kpack + a
                    ws = wt[:, a * D:(a + 1) * D]
                    sc2 = ascp.tile([CHUNK, D], BF16, tag="sc2")
                    nc.scalar.activation(
                        out=sc2[:], in_=ws, func=AF.Square,
                        accum_out=v[:, col:col + 1],
                    )
                # rows [na, kpack): one wide ACT square, then DVE pairwise
                # fold (2x bf16) + one 3D reduce into contiguous v columns.
                w2 = w2pool.tile([CHUNK, nd * D], BF16, tag="w2")
                nc.scalar.activation(
                    out=w2[:], in_=wt[:, na * D:kpack * D], func=AF.Square,
                )
                w23 = w2[:].rearrange("p (k d) -> p k d", k=nd)
                f = fpool.tile([CHUNK, nd * (D // 2)], BF16, tag="f")
                f3 = f[:].rearrange("p (k d) -> p k d", k=nd)
                nc.vector.tensor_add(
                    f3, w23[:, :, 0:D // 2], w23[:, :, D // 2:D]
                )
                g = fpool.tile([CHUNK, nd * (D // 4)], BF16, tag="g")
                g3 = g[:].rearrange("p (k d) -> p k d", k=nd)
                nc.vector.tensor_add(
                    g3, f3[:, :, 0:D // 4], f3[:, :, D // 4:D // 2]
                )
                nc.vector.tensor_reduce(
                    v[:, j * kpack + na:(j + 1) * kpack], g3,
                    axis=mybir.AxisListType.X, op=OP.add,
                )

            # Phase B: dist via NR sqrt; p = exp(SHIFT - dist) in bf16.
            y0 = stats.tile([128, nchunk], F32, tag="y0")
            nc.vector.tensor_scalar(
                out=y0[:], in0=v[:], scalar1=FIT_B, scalar2=FIT_A,
                op0=OP.mult, op1=OP.add,
            )
            ycur = y0
            for it in range(2):
                r = stats.tile([128, nchunk], F32, tag=f"r{it}")
                nc.vector.reciprocal(r[:], ycur[:])
                t = stats.tile([128, nchunk], F32, tag=f"t{it}")
                nc.vector.tensor_mul(t[:], v[:], r[:])
                u = stats.tile([128, nchunk], F32, tag=f"u{it}")
                nc.vector.tensor_add(u[:], ycur[:], t[:])
                ynext = stats.tile([128, nchunk], F32, tag=f"y{it + 1}")
                nc.vector.tensor_scalar(
                    out=ynext[:], in0=u[:], scalar1=0.5, scalar2=None, op0=OP.mult,
                )
                ycur = ynext

            p = stats.tile([128, nchunk], BF16, tag="p")
            nc.scalar.activation(
                out=p[:], in_=ycur[:], func=AF.Exp, scale=-1.0, bias=shiftt[:],
            )

            # Phase C: psum[0, 0:100] = sum_s p_s * L[s, :], psum[0, 100] = Z.
            acc = psum.tile([1, CA], F32, tag="acc")
            for j in range(nblk):
                lt = lpool.tile([CHUNK, kpack * CA], BF16, tag="lt")
                nc.sync.dma_start(lt[:], Lb[j])
                for a in range(kpack):
                    col = j * kpack + a
                    nc.tensor.matmul(
                        acc[:], p[:, col:col + 1], lt[:, a * CA:(a + 1) * CA],
                        start=(col == 0), stop=(col == nchunk - 1),
                    )

            rz = stats.tile([1, 1], F32, tag="rz")
            nc.vector.reciprocal(rz[:], acc[0:1, C:C + 1])
            ob = outp.tile([1, C], F32, tag="ob")
            nc.vector.tensor_scalar(
                out=ob[:], in0=acc[0:1, 0:C], scalar1=rz[:], scalar2=None,
                op0=OP.mult,
            )
            nc.sync.dma_start(OUT[b:b + 1, :], ob[:])

    nc.finalize()
    return nc


_NC_CACHE = []
LAST_RESULT = None
BF = ml_dtypes.bfloat16


def _prep_core(q, X, L):
    """Host-side prep for one core's slice: bf16 casts, ones column on L,
    q broadcast."""
    bpc = q.shape[0]
    s = X.shape[1]
    Laug = np.empty((bpc, s, CA), dtype=BF)
    Laug[:, :, :C] = L
    Laug[:, :, C] = 1.0
    qr = np.ascontiguousarray(
        np.broadcast_to(q.astype(BF)[:, None, :], (bpc, 128, D))
    )
    return {"x": X.astype(BF), "l": Laug, "qr": qr}


def kernel(**inputs) -> np.ndarray:
    global LAST_RESULT
    q = np.asarray(inputs["query_feats"], dtype=np.float32)
    X = np.asarray(inputs["support_feats"], dtype=np.float32)
    L = np.asarray(inputs["support_labels"], dtype=np.float32)
    assert q.shape == (B, D) and X.shape == (B, S, D) and L.shape == (B, S, C)

    if not _NC_CACHE:
        _NC_CACHE.append(_build_nc())
    nc = _NC_CACHE[0]

    in_maps = []
    for c in range(NCORES):
        sl = slice(c * BPC, (c + 1) * BPC)
        in_maps.append(_prep_core(q[sl], X[sl], L[sl]))

    res = run_bass_kernel_spmd(nc, in_maps, list(range(NCORES)))
    LAST_RESULT = res
    out = np.concatenate([res.results[c]["out"] for c in range(NCORES)], axis=0)
    return out.astype(np.float32)


# revision 18
# speedup vs baseline: 4.1291x; 1.0003x over previous
"""Nadaraya-Watson head (retrieval kNN) Trainium2 Bass kernel.

reference:
    dist = ||q - x||_2 over d            (b, s)
    probs = softmax(-dist, axis=s)       (b, s)
    out = probs @ labels                 (b, c)

Strategy (8 NeuronCores, batch-parallel, 8 batches per core):
  All big operands are bf16 (host-cast): halves HBM traffic (the
  memory-bound term), enables DVE 2x packing, single-pass PE matmuls.
  Reductions/accumulations stay fp32 (PSUM, accum_out, stats math).

  dist^2 = sum_d (x - q)^2 computed in natural [s=partition, d=free]
  layout (no transposes):
    - one wide DVE tensor_sub per [128, kpack*D] tile: W = X - q, with q
      read through a stride-0 broadcast AP (2x bf16 mode)
    - per-row sum W^2 via activation(Square, accum_out) on ACT and
      scalar_tensor_tensor(W*W, accum_out) on DVE, split to balance the
      two engines (DVE_SQ_N of every kpack go to DVE)
  dist = sqrt via linear seed + 2 Newton-Raphson steps on DVE (avoids the
  sqrt ACT table set; Square/Exp share the exp_and_others set -> one table
  load total).
  Softmax shift is a constant (exact math; dist concentrates near 22.6 so
  exp stays in range without a max pass).
  Label reduction: PE matmul with the bf16 probs column [128,1] stationary
  and the label tile [128,101] moving (col 100 = host-appended ones column,
  which makes the softmax normalizer Z fall out of the same fp32 PSUM
  accumulation). Final scale by 1/Z on DVE.

  DMA layout: kpack=16 consecutive support rows packed per SBUF partition
  -> 8KB (X) / 3.2KB (L) per-partition DMA lines, needed for full HBM
  bandwidth. Score column j*kpack + a <-> support row
  128*kpack*j + kpack*p + a on partition p; the label matmul consumes the
  matching L sub-slice, so ordering stays consistent.
"""

from contextlib import ExitStack

import ml_dtypes
import numpy as np

import concourse.bacc as bacc
import concourse.tile as tile
from concourse import mybir
from concourse.bass_utils import run_bass_kernel_spmd

F32 = mybir.dt.float32
BF16 = mybir.dt.bfloat16
OP = mybir.AluOpType
AF = mybir.ActivationFunctionType

# Problem sizes (hardcoded per harness contract).
B, S, D, C = 64, 8192, 256, 100
CA = C + 1                 # labels + ones column
NCORES = 8
BPC = B // NCORES          # batches per core
CHUNK = 128                # support rows per tile (partition dim)

# Constant softmax shift: exp(SHIFT - dist). Exact math (softmax is
# shift-invariant); dist concentrates near sqrt(2*D) ~ 22.6.
SHIFT = 22.0

# Minimax linear seed for sqrt(v) on v in [250, 900] (dist^2 range with huge
# margin), refined by two Newton-Raphson steps -> rel err ~1e-7.
FIT_B = 0.0218287
FIT_A = 10.9031


def _build_nc(bpc=BPC, s=S, kpack=32, act_accum_n=8):
    """kpack: consecutive support rows per SBUF partition (DMA line size).
    Of every kpack rows, the first act_accum_n get ACT accum-squares; the
    rest go through ACT wide-square -> DVE pair-fold -> DVE 3D reduce."""
    nchunk = s // CHUNK           # score columns per batch
    nblk = s // (CHUNK * kpack)   # DMA tiles per batch
    na = act_accum_n
    nd = kpack - na               # rows per tile on the fold+reduce path
    nc = bacc.Bacc(None)
    X = nc.declare_dram_parameter("x", [bpc, s, D], BF16, isOutput=False)
    L = nc.declare_dram_parameter("l", [bpc, s, CA], BF16, isOutput=False)
    QR = nc.declare_dram_parameter("qr", [bpc, 128, D], BF16, isOutput=False)
    OUT = nc.declare_dram_parameter("out", [bpc, C], F32, isOutput=True)

    with tile.TileContext(nc) as tc, ExitStack() as ctx:
        xpool = ctx.enter_context(tc.tile_pool(name="xpool", bufs=4))
        lpool = ctx.enter_context(tc.tile_pool(name="lpool", bufs=4))
        wpool = ctx.enter_context(tc.tile_pool(name="wpool", bufs=2))
        w2pool = ctx.enter_context(tc.tile_pool(name="w2pool", bufs=2))
        fpool = ctx.enter_context(tc.tile_pool(name="fpool", bufs=2))
        ascp = ctx.enter_context(tc.tile_pool(name="ascp", bufs=2))
        qpool = ctx.enter_context(tc.tile_pool(name="qpool", bufs=2))
        stats = ctx.enter_context(tc.tile_pool(name="stats", bufs=2))
        outp = ctx.enter_context(tc.tile_pool(name="outp", bufs=2))
        cons = ctx.enter_context(tc.tile_pool(name="cons", bufs=1))
        psum = ctx.enter_context(tc.tile_pool(name="psum", bufs=2, space="PSUM"))

        shiftt = cons.tile([128, 1], F32)
        nc.vector.memset(shiftt[:], SHIFT)

        for b in range(bpc):
            Xb = X[b].rearrange("(n p k) d -> n p (k d)", p=CHUNK, k=kpack)
            Lb = L[b].rearrange("(n p k) c -> n p (k c)", p=CHUNK, k=kpack)

            qr = qpool.tile([128, D], BF16, tag="qr")
            nc.sync.dma_start(qr[:], QR[b])
            qbc = qr[:].rearrange("p (k d) -> p k d", k=1).to_broadcast(
                (128, kpack, D)
            )

            v = stats.tile([128, nchunk], F32, tag="v")

            # Phase A: stream X; W = X - q; v[:, col] = sum_d W^2 = dist^2.
            for j in range(nblk):
                xt = xpool.tile([CHUNK, kpack * D], BF16, tag="xt")
                nc.sync.dma_start(xt[:], Xb[j])
                wt = wpool.tile([CHUNK, kpack * D], BF16, tag="wt")
                nc.vector.tensor_sub(
                    wt[:].rearrange("p (k d) -> p k d", k=kpack),
                    xt[:].rearrange("p (k d) -> p k d", k=kpack),
                    qbc,
                )
                # rows [0, na): ACT square with per-row accumulate
                for a in range(na):
                    col = j * kpack + a
                    ws = wt[:, a * D:(a + 1) * D]
                    sc2 = ascp.tile([CHUNK, D], BF16, tag="sc2")
                    nc.scalar.activation(
                        out=sc2[:], in_=ws, func=AF.Square,
                        accum_out=v[:, col:col + 1],
                    )
                # rows [na, kpack): one wide ACT square, then DVE pairwise
                # fold (2x bf16) + one 3D reduce into contiguous v columns.
                w2 = w2pool.tile([CHUNK, nd * D], BF16, tag="w2")
                nc.scalar.activation(
                    out=w2[:], in_=wt[:, na * D:kpack * D], func=AF.Square,
                )
                w23 = w2[:].rearrange("p (k d) -> p k d", k=nd)
                f = fpool.tile([CHUNK, nd * (D // 2)], BF16, tag="f")
                f3 = f[:].rearrange("p (k d) -> p k d", k=nd)
                nc.vector.tensor_add(
                    f3, w23[:, :, 0:D // 2], w23[:, :, D // 2:D]
                )
                g = fpool.tile([CHUNK, nd * (D // 4)], BF16, tag="g")
                g3 = g[:].rearrange("p (k d) -> p k d", k=nd)
                nc.vector.tensor_add(
                    g3, f3[:, :, 0:D // 4], f3[:, :, D // 4:D // 2]
                )
                h = fpool.tile([CHUNK, nd * (D // 8)], BF16, tag="h")
                h3 = h[:].rearrange("p (k d) -> p k d", k=nd)
                nc.vector.tensor_add(
                    h3, g3[:, :, 0:D // 8], g3[:, :, D // 8:D // 4]
                )
                nc.vector.tensor_reduce(
                    v[:, j * kpack + na:(j + 1) * kpack], h3,
                    axis=mybir.AxisListType.X, op=OP.add,
                )

            # Phase B: dist via NR sqrt; p = exp(SHIFT - dist) in bf16.
            y0 = stats.tile([128, nchunk], F32, tag="y0")
            nc.vector.tensor_scalar(
                out=y0[:], in0=v[:], scalar1=FIT_B, scalar2=FIT_A,
                op0=OP.mult, op1=OP.add,
            )
            ycur = y0
            for it in range(2):
                r = stats.tile([128, nchunk], F32, tag=f"r{it}")
                nc.vector.reciprocal(r[:], ycur[:])
                t = stats.tile([128, nchunk], F32, tag=f"t{it}")
                nc.vector.tensor_mul(t[:], v[:], r[:])
                u = stats.tile([128, nchunk], F32, tag=f"u{it}")
                nc.vector.tensor_add(u[:], ycur[:], t[:])
                ynext = stats.tile([128, nchunk], F32, tag=f"y{it + 1}")
                nc.vector.tensor_scalar(
                    out=ynext[:], in0=u[:], scalar1=0.5, scalar2=None, op0=OP.mult,
                )
                ycur = ynext

            p = stats.tile([128, nchunk], BF16, tag="p")
            nc.scalar.activation(
                out=p[:], in_=ycur[:], func=AF.Exp, scale=-1.0, bias=shiftt[:],
            )

            # Phase C: psum[0, 0:100] = sum_s p_s * L[s, :], psum[0, 100] = Z.
            acc = psum.tile([1, CA], F32, tag="acc")
            for j in range(nblk):
                lt = lpool.tile([CHUNK, kpack * CA], BF16, tag="lt")
                nc.sync.dma_start(lt[:], Lb[j])
                for a in range(kpack):
                    col = j * kpack + a
                    nc.tensor.matmul(
                        acc[:], p[:, col:col + 1], lt[:, a * CA:(a + 1) * CA],
                        start=(col == 0), stop=(col == nchunk - 1),
                    )

            rz = stats.tile([1, 1], F32, tag="rz")
            nc.vector.reciprocal(rz[:], acc[0:1, C:C + 1])
            ob = outp.tile([1, C], F32, tag="ob")
            nc.vector.tensor_scalar(
                out=ob[:], in0=acc[0:1, 0:C], scalar1=rz[:], scalar2=None,
                op0=OP.mult,
            )
            nc.sync.dma_start(OUT[b:b + 1, :], ob[:])

    nc.finalize()
    return nc


_NC_CACHE = []
LAST_RESULT = None
BF = ml_dtypes.bfloat16


def _prep_core(q, X, L):
    """Host-side prep for one core's slice: bf16 casts, ones column on L,
    q broadcast."""
    bpc = q.shape[0]
    s = X.shape[1]
    Laug = np.empty((bpc, s, CA), dtype=BF)
    Laug[:, :, :C] = L
    Laug[:, :, C] = 1.0
    qr = np.ascontiguousarray(
        np.broadcast_to(q.astype(BF)[:, None, :], (bpc, 128, D))
    )
    return {"x": X.astype(BF), "l": Laug, "qr": qr}


def kernel(**inputs) -> np.ndarray:
    global LAST_RESULT
    q = np.asarray(inputs["query_feats"], dtype=np.float32)
    X = np.asarray(inputs["support_feats"], dtype=np.float32)
    L = np.asarray(inputs["support_labels"], dtype=np.float32)
    assert q.shape == (B, D) and X.shape == (B, S, D) and L.shape == (B, S, C)

    if not _NC_CACHE:
        _NC_CACHE.append(_build_nc())
    nc = _NC_CACHE[0]

    in_maps = []
    for c in range(NCORES):
        sl = slice(c * BPC, (c + 1) * BPC)
        in_maps.append(_prep_core(q[sl], X[sl], L[sl]))

    res = run_bass_kernel_spmd(nc, in_maps, list(range(NCORES)))
    LAST_RESULT = res
    out = np.concatenate([res.results[c]["out"] for c in range(NCORES)], axis=0)
    return out.astype(np.float32)


# revision 22
# speedup vs baseline: 4.2382x; 1.0264x over previous
"""Nadaraya-Watson head (retrieval kNN) Trainium2 Bass kernel.

reference:
    dist = ||q - x||_2 over d            (b, s)
    probs = softmax(-dist, axis=s)       (b, s)
    out = probs @ labels                 (b, c)

Strategy (8 NeuronCores, batch-parallel, 8 batches per core):
  All big operands are bf16 (host-cast): halves HBM traffic (the
  memory-bound term), enables DVE 2x packing, single-pass PE matmuls.
  Reductions/accumulations stay fp32 (PSUM, accum_out, stats math).

  dist^2 = sum_d (x - q)^2 computed in natural [s=partition, d=free]
  layout (no transposes):
    - one wide DVE tensor_sub per [128, kpack*D] tile: W = X - q, with q
      read through a stride-0 broadcast AP (2x bf16 mode)
    - per-row sum W^2 via activation(Square, accum_out) on ACT and
      scalar_tensor_tensor(W*W, accum_out) on DVE, split to balance the
      two engines (DVE_SQ_N of every kpack go to DVE)
  dist = sqrt via linear seed + 2 Newton-Raphson steps on DVE (avoids the
  sqrt ACT table set; Square/Exp share the exp_and_others set -> one table
  load total).
  Softmax shift is a constant (exact math; dist concentrates near 22.6 so
  exp stays in range without a max pass).
  Label reduction: PE matmul with the bf16 probs column [128,1] stationary
  and the label tile [128,101] moving (col 100 = host-appended ones column,
  which makes the softmax normalizer Z fall out of the same fp32 PSUM
  accumulation). Final scale by 1/Z on DVE.

  DMA layout: kpack=16 consecutive support rows packed per SBUF partition
  -> 8KB (X) / 3.2KB (L) per-partition DMA lines, needed for full HBM
  bandwidth. Score column j*kpack + a <-> support row
  128*kpack*j + kpack*p + a on partition p; the label matmul consumes the
  matching L sub-slice, so ordering stays consistent.
"""

from contextlib import ExitStack

import ml_dtypes
import numpy as np

import concourse.bacc as bacc
import concourse.tile as tile
from concourse import mybir
from concourse.bass_utils import run_bass_kernel_spmd

F32 = mybir.dt.float32
BF16 = mybir.dt.bfloat16
OP = mybir.AluOpType
AF = mybir.ActivationFunctionType

# Problem sizes (hardcoded per harness contract).
B, S, D, C = 64, 8192, 256, 100
CA = C + 1                 # labels + ones column
NCORES = 8
BPC = B // NCORES          # batches per core
CHUNK = 128                # support rows per tile (partition dim)

# Constant softmax shift: exp(SHIFT - dist). Exact math (softmax is
# shift-invariant); dist concentrates near sqrt(2*D) ~ 22.6.
SHIFT = 22.0

# Minimax linear seed for sqrt(v) on v in [250, 900] (dist^2 range with huge
# margin), refined by two Newton-Raphson steps -> rel err ~1e-7.
FIT_B = 0.0218287
FIT_A = 10.9031


def _build_nc(bpc=BPC, s=S, kpack=32, act_accum_n=3):
    """kpack: consecutive support rows per SBUF partition (DMA line size).
    Of every kpack rows, the first act_accum_n get ACT accum-squares; the
    rest go through ACT wide-square -> DVE pair-fold -> DVE 3D reduce."""
    nchunk = s // CHUNK           # score columns per batch
    nblk = s // (CHUNK * kpack)   # DMA tiles per batch
    na = act_accum_n
    nd = kpack - na               # rows per tile on the fold+reduce path
    nc = bacc.Bacc(None)
    X = nc.declare_dram_parameter("x", [bpc, s, D], BF16, isOutput=False)
    L = nc.declare_dram_parameter("l", [bpc, s, CA], BF16, isOutput=False)
    QR = nc.declare_dram_parameter("qr", [bpc, 128, D], BF16, isOutput=False)
    OUT = nc.declare_dram_parameter("out", [bpc, C], F32, isOutput=True)

    with tile.TileContext(nc) as tc, ExitStack() as ctx:
        xpool = ctx.enter_context(tc.tile_pool(name="xpool", bufs=4))
        lpool = ctx.enter_context(tc.tile_pool(name="lpool", bufs=4))
        wpool = ctx.enter_context(tc.tile_pool(name="wpool", bufs=2))
        w2pool = ctx.enter_context(tc.tile_pool(name="w2pool", bufs=2))
        fpool = ctx.enter_context(tc.tile_pool(name="fpool", bufs=2))
        ascp = ctx.enter_context(tc.tile_pool(name="ascp", bufs=2))
        qpool = ctx.enter_context(tc.tile_pool(name="qpool", bufs=2))
        stats = ctx.enter_context(tc.tile_pool(name="stats", bufs=2))
        outp = ctx.enter_context(tc.tile_pool(name="outp", bufs=2))
        cons = ctx.enter_context(tc.tile_pool(name="cons", bufs=1))
        psum = ctx.enter_context(tc.tile_pool(name="psum", bufs=2, space="PSUM"))

        shiftt = cons.tile([128, 1], F32)
        nc.vector.memset(shiftt[:], SHIFT)

        for b in range(bpc):
            Xb = X[b].rearrange("(n p k) d -> n p (k d)", p=CHUNK, k=kpack)
            Lb = L[b].rearrange("(n p k) c -> n p (k c)", p=CHUNK, k=kpack)

            qr = qpool.tile([128, D], BF16, tag="qr")
            nc.sync.dma_start(qr[:], QR[b])
            qbc = qr[:].rearrange("p (k d) -> p k d", k=1).to_broadcast(
                (128, kpack, D)
            )

            v = stats.tile([128, nchunk], F32, tag="v")

            # Phase A: stream X; W = X - q; v[:, col] = sum_d W^2 = dist^2.
            for j in range(nblk):
                xt = xpool.tile([CHUNK, kpack * D], BF16, tag="xt")
                nc.sync.dma_start(xt[:], Xb[j])
                wt = wpool.tile([CHUNK, kpack * D], BF16, tag="wt")
                nc.vector.tensor_sub(
                    wt[:].rearrange("p (k d) -> p k d", k=kpack),
                    xt[:].rearrange("p (k d) -> p k d", k=kpack),
                    qbc,
                )
                # rows [0, na): ACT square with per-row accumulate
                for a in range(na):
                    col = j * kpack + a
                    ws = wt[:, a * D:(a + 1) * D]
                    sc2 = ascp.tile([CHUNK, D], BF16, tag="sc2")
                    nc.scalar.activation(
                        out=sc2[:], in_=ws, func=AF.Square,
                        accum_out=v[:, col:col + 1],
                    )
                # rows [na, kpack): one wide ACT square, then DVE pairwise
                # fold (2x bf16) + one 3D reduce into contiguous v columns.
                w2 = w2pool.tile([CHUNK, nd * D], BF16, tag="w2")
                nc.scalar.activation(
                    out=w2[:], in_=wt[:, na * D:kpack * D], func=AF.Square,
                )
                w23 = w2[:].rearrange("p (k d) -> p k d", k=nd)
                f = fpool.tile([CHUNK, nd * (D // 2)], BF16, tag="f")
                f3 = f[:].rearrange("p (k d) -> p k d", k=nd)
                nc.vector.tensor_add(
                    f3, w23[:, :, 0:D // 2], w23[:, :, D // 2:D]
                )
                g = fpool.tile([CHUNK, nd * (D // 4)], BF16, tag="g")
                g3 = g[:].rearrange("p (k d) -> p k d", k=nd)
                nc.vector.tensor_add(
                    g3, f3[:, :, 0:D // 4], f3[:, :, D // 4:D // 2]
                )
                h = fpool.tile([CHUNK, nd * (D // 8)], BF16, tag="h")
                h3 = h[:].rearrange("p (k d) -> p k d", k=nd)
                nc.vector.tensor_add(
                    h3, g3[:, :, 0:D // 8], g3[:, :, D // 8:D // 4]
                )
                nc.vector.tensor_reduce(
                    v[:, j * kpack + na:(j + 1) * kpack], h3,
                    axis=mybir.AxisListType.X, op=OP.add,
                )

            # Phase B: dist via NR sqrt; p = exp(SHIFT - dist) in bf16.
            y0 = stats.tile([128, nchunk], F32, tag="y0")
            nc.vector.tensor_scalar(
                out=y0[:], in0=v[:], scalar1=FIT_B, scalar2=FIT_A,
                op0=OP.mult, op1=OP.add,
            )
            ycur = y0
            for it in range(2):
                r = stats.tile([128, nchunk], F32, tag=f"r{it}")
                nc.vector.reciprocal(r[:], ycur[:])
                t = stats.tile([128, nchunk], F32, tag=f"t{it}")
                nc.vector.tensor_mul(t[:], v[:], r[:])
                u = stats.tile([128, nchunk], F32, tag=f"u{it}")
                nc.vector.tensor_add(u[:], ycur[:], t[:])
                ynext = stats.tile([128, nchunk], F32, tag=f"y{it + 1}")
                nc.vector.tensor_scalar(
                    out=ynext[:], in0=u[:], scalar1=0.5, scalar2=None, op0=OP.mult,
                )
                ycur = ynext

            p = stats.tile([128, nchunk], BF16, tag="p")
            nc.scalar.activation(
                out=p[:], in_=ycur[:], func=AF.Exp, scale=-1.0, bias=shiftt[:],
            )

            # Phase C: acc_g[0, 0:100] = partial sum_s p_s * L[s, :],
            # acc_g[0, 100] = partial Z. Round-robin over NBANK PSUM banks so
            # consecutive matmuls hit different banks and their pipeline
            # drains overlap (same-bank accumulation serializes at the
            # isolated-matmul latency).
            NBANK = 4
            accs = [
                psum.tile([1, CA], F32, tag=f"acc{g}", name=f"acc{g}")
                for g in range(NBANK)
            ]
            for j in range(nblk):
                lt = lpool.tile([CHUNK, kpack * CA], BF16, tag="lt")
                nc.sync.dma_start(lt[:], Lb[j])
                for a in range(kpack):
                    col = j * kpack + a
                    nc.tensor.matmul(
                        accs[col % NBANK][:], p[:, col:col + 1],
                        lt[:, a * CA:(a + 1) * CA],
                        start=(col < NBANK), stop=(col >= nchunk - NBANK),
                    )

            c0 = outp.tile([1, CA], F32, tag="c0")
            nc.vector.tensor_copy(c0[:], accs[0][:])
            c1 = outp.tile([1, CA], F32, tag="c1")
            nc.vector.tensor_add(c1[:], c0[:], accs[1][:])
            c2 = outp.tile([1, CA], F32, tag="c2")
            nc.vector.tensor_add(c2[:], c1[:], accs[2][:])
            stot = outp.tile([1, CA], F32, tag="stot")
            nc.vector.tensor_add(stot[:], c2[:], accs[3][:])

            rz = stats.tile([1, 1], F32, tag="rz")
            nc.vector.reciprocal(rz[:], stot[0:1, C:C + 1])
            ob = outp.tile([1, C], F32, tag="ob")
            nc.vector.tensor_scalar(
                out=ob[:], in0=stot[0:1, 0:C], scalar1=rz[:], scalar2=None,
                op0=OP.mult,
            )
            nc.sync.dma_start(OUT[b:b + 1, :], ob[:])

    nc.finalize()
    return nc


_NC_CACHE = []
LAST_RESULT = None
BF = ml_dtypes.bfloat16


def _prep_core(q, X, L):
    """Host-side prep for one core's slice: bf16 casts, ones column on L,
    q broadcast."""
    bpc = q.shape[0]
    s = X.shape[1]
    Laug = np.empty((bpc, s, CA), dtype=BF)
    Laug[:, :, :C] = L
    Laug[:, :, C] = 1.0
    qr = np.ascontiguousarray(
        np.broadcast_to(q.astype(BF)[:, None, :], (bpc, 128, D))
    )
    return {"x": X.astype(BF), "l": Laug, "qr": qr}


def kernel(**inputs) -> np.ndarray:
    global LAST_RESULT
    q = np.asarray(inputs["query_feats"], dtype=np.float32)
    X = np.asarray(inputs["support_feats"], dtype=np.float32)
    L = np.asarray(inputs["support_labels"], dtype=np.float32)
    assert q.shape == (B, D) and X.shape == (B, S, D) and L.shape == (B, S, C)

    if not _NC_CACHE:
        _NC_CACHE.append(_build_nc())
    nc = _NC_CACHE[0]

    in_maps = []
    for c in range(NCORES):
        sl = slice(c * BPC, (c + 1) * BPC)
        in_maps.append(_prep_core(q[sl], X[sl], L[sl]))

    res = run_bass_kernel_spmd(nc, in_maps, list(range(NCORES)))
    LAST_RESULT = res
    out = np.concatenate([res.results[c]["out"] for c in range(NCORES)], axis=0)
    return out.astype(np.float32)
